# revision 14
# baseline (speedup 1.0000x reference)
"""Bahdanau attention on 8 TRN2 NeuronCores (Bass/Tile).

B=16, S=4096, D=256, U=256. Data-parallel over batch: core i handles
batches [2i, 2i+2). Weights replicated.

Per-core pipeline per batch:
  1. DMA values[b] (4 MB) -> SBUF natural layout [128(s), 32(t), 256(d)].
  2. PE-transpose 128x128 blocks -> vT [128(d), s] chunks (fp32, via PSUM +
     DVE copy to SBUF).
  3. proj^T[u, s] = W1^T @ vT (fp32r matmuls, PSUM accumulate over d-halves).
  4. ACT: tanh(proj + qproj + b1 + b2) with per-partition bias (u on
     partitions), PSUM -> SBUF.
  5. score[1, s] = V^T @ tanh (fp32r), landed as PSUM rows [8, 512].
  6. Softmax without max-shift (|score| <= ||V||_1 ~ 13, exp is safe in
     fp32): ACT exp with fused row-sum, PE all-ones matmul for the
     cross-partition sum, DVE reciprocal + scale.
  7. attention weights DMA'd out; PE-transposed into per-s-tile columns
     [128, 32] for the weighted sum.
  8. context[1, 256] = sum_t p_t^T @ v_t (fp32r, PSUM accumulate), DMA out.
"""

import os
import sys
from contextlib import ExitStack

import numpy as np

if "/opt/trn_rl_repo" not in sys.path:
    sys.path.insert(0, "/opt/trn_rl_repo")

import concourse.bacc as bacc
import concourse.bass as bass
import concourse.tile as tile
from concourse import mybir
from concourse.bass_utils import run_bass_kernel_spmd
from concourse.masks import make_identity

B, S, D, U = 16, 4096, 256, 256
NCORES = 8
BPC = B // NCORES  # batches per core
P = 128
NT = S // P       # 32 s-tiles per batch
NCH = S // 512    # 8 chunks of 512 s
F32 = mybir.dt.float32
F32R = mybir.dt.float32r
AF = mybir.ActivationFunctionType

# Set by test.py to capture a profile.
TRACE = False
TRACE_DIR = "/tmp/bahdanau_trace"
LAST_EXEC_NS = None

_CACHED = {}


def _r(ap):
    return ap.bitcast(F32R)


def _body(ctx, tc, values, query, W1, b1, W2, b2, V, ctxO, attwO):
    nc = tc.nc
    consts = ctx.enter_context(tc.tile_pool(name="consts", bufs=1))
    natp = ctx.enter_context(tc.tile_pool(name="nat", bufs=2))
    vtp = ctx.enter_context(tc.tile_pool(name="vts", bufs=3))
    thp = ctx.enter_context(tc.tile_pool(name="tanh", bufs=3))
    smp = ctx.enter_context(tc.tile_pool(name="smx", bufs=2))
    ps_vt = ctx.enter_context(tc.tile_pool(name="psvt", bufs=2, space="PSUM"))
    ps_pj = ctx.enter_context(tc.tile_pool(name="pspj", bufs=2, space="PSUM"))
    ps_sc = ctx.enter_context(tc.tile_pool(name="pssc", bufs=2, space="PSUM"))
    ps_ms = ctx.enter_context(tc.tile_pool(name="psms", bufs=2, space="PSUM"))

    # ---- constants / preload ----
    # Everything the PE reads (except the big values DMA) is funneled
    # through a single DVE copy: PE matmuls carry at most ONE semaphore
    # wait (the fused-LDWEIGHTS struct only supports one), and all these
    # collapse onto the DVE clock.
    ident_st = consts.tile([P, P], F32)
    make_identity(nc, ident_st)
    w1_st = consts.tile([P, 2, U], F32)
    nc.sync.dma_start(out=w1_st, in_=W1.rearrange("(h p) u -> p h u", p=P))
    w2_st = consts.tile([P, 2, U], F32)
    nc.sync.dma_start(out=w2_st, in_=W2.rearrange("(h p) u -> p h u", p=P))
    v_st = consts.tile([P, 2], F32)
    nc.sync.dma_start(
        out=v_st, in_=V.rearrange("(h p) one -> p h one", p=P)[:, :, 0]
    )
    q_st = consts.tile([P, 2, BPC], F32)
    qr_ = query.rearrange("b (h p) -> p h b", p=P)
    for h in range(2):
        nc.sync.dma_start(out=q_st[:, h, :], in_=qr_[:, h, :])
    b1_st = consts.tile([P, 2], F32)
    nc.sync.dma_start(out=b1_st, in_=b1.rearrange("(h p) -> p h", p=P))
    b2_st = consts.tile([P, 2], F32)
    nc.sync.dma_start(out=b2_st, in_=b2.rearrange("(h p) -> p h", p=P))

    ident = consts.tile([P, P], F32R)
    nc.vector.tensor_copy(ident, ident_st)
    ident1 = consts.tile([1, 1], F32)
    nc.vector.memset(ident1, 1.0)
    ones1 = consts.tile([1, P], F32)
    nc.vector.memset(ones1, 1.0)
    w1_sb = consts.tile([P, 2, U], F32R)
    nc.vector.tensor_copy(w1_sb, w1_st)
    v_sb = consts.tile([P, 2], F32R)
    nc.vector.tensor_copy(v_sb, v_st)
    w2_sb = consts.tile([P, 2, U], F32)
    nc.vector.tensor_copy(w2_sb, w2_st)
    q_sb = consts.tile([P, 2, BPC], F32)
    for h in range(2):
        nc.vector.tensor_copy(q_sb[:, h, :], q_st[:, h, :])
    b1_sb = consts.tile([P, 2], F32)
    nc.vector.tensor_copy(b1_sb, b1_st)
    b2_sb = consts.tile([P, 2], F32)
    nc.vector.tensor_copy(b2_sb, b2_st)

    # tanh bias per (u, b): qproj[u, b] + b1[u] + b2[u]
    bias_sb = consts.tile([P, 2, BPC], F32)
    for uh in range(2):
        qp_ps = ps_ms.tile([P, BPC], F32, tag="ms")
        for dh in range(2):
            nc.tensor.matmul(
                qp_ps,
                w2_sb[:, dh, uh * P : (uh + 1) * P],
                q_sb[:, dh, :],
                start=(dh == 0),
                stop=(dh == 1),
            )
        for b in range(BPC):
            nc.vector.tensor_add(
                bias_sb[:, uh, b : b + 1], qp_ps[:, b : b + 1], b1_sb[:, uh : uh + 1]
            )
            nc.vector.tensor_add(
                bias_sb[:, uh, b : b + 1],
                bias_sb[:, uh, b : b + 1],
                b2_sb[:, uh : uh + 1],
            )

    for b in range(BPC):
        nat = natp.tile([P, NT, D], F32R)
        vr = values[b].rearrange("(t p) d -> p t d", p=P)
        for h in range(2):
            nc.sync.dma_start(
                out=nat[:, h * 16 : (h + 1) * 16, :], in_=vr[:, h * 16 : (h + 1) * 16, :]
            )

        exp_sb = smp.tile([1, NCH, 512], F32)
        zrow = smp.tile([1, NCH], F32)
        for c in range(NCH):
            # ---- transpose 512 s-values x 256 d into vT chunk ----
            vt_sb = vtp.tile([P, 2, 512], F32R)
            for q in range(2):
                vt_ps = ps_vt.tile([P, 2, 256], F32R)
                for dh in range(2):
                    for j in range(2):
                        t = 4 * c + 2 * q + j
                        nc.tensor.transpose(
                            vt_ps[:, dh, j * P : (j + 1) * P],
                            nat[:, t, dh * P : (dh + 1) * P],
                            ident,
                        )
                nc.vector.tensor_copy(vt_sb[:, :, q * 256 : (q + 1) * 256], vt_ps)

            # ---- proj + tanh + score for this chunk ----
            score_ps = ps_sc.tile([1, 512], F32)
            for uh in range(2):
                pj = ps_pj.tile([P, 512], F32)
                for dh in range(2):
                    nc.tensor.matmul(
                        pj,
                        w1_sb[:, dh, uh * P : (uh + 1) * P],
                        vt_sb[:, dh, :],
                        start=(dh == 0),
                        stop=(dh == 1),
                    )
                th = thp.tile([P, 512], F32R)
                nc.scalar.activation(th, pj, AF.Tanh, bias=bias_sb[:, uh, b : b + 1])
                nc.tensor.matmul(
                    score_ps,
                    v_sb[:, uh : uh + 1],
                    _r(th),
                    start=(uh == 0),
                    stop=(uh == 1),
                )
            # exp with fused row-sum; no max-shift (scores bounded by ||V||_1)
            nc.scalar.activation(
                exp_sb[:, c, :],
                score_ps,
                AF.Exp,
                accum_out=zrow[:, c : c + 1],
            )
        # Z = sum of row sums; 1/Z broadcast to all 128 partitions via k=1
        # all-ones matmul.
        z_sb = smp.tile([1, 1], F32)
        nc.vector.reduce_sum(z_sb, zrow, axis=mybir.AxisListType.X)
        zb_ps = ps_ms.tile([P, 1], F32, tag="ms")
        nc.tensor.matmul(zb_ps, ones1[:1, :], z_sb, start=True, stop=True)
        r128 = smp.tile([P, 1], F32)
        nc.vector.reciprocal(r128, zb_ps)

        # ---- exp -> per-s-tile columns [128, 32] (transpose), normalize ----
        pc_ps = ps_ms.tile([P, 32], F32, tag="ms")
        for t in range(NT):
            nc.tensor.transpose(
                pc_ps[:, t : t + 1],
                exp_sb[:, t // 4, (t % 4) * P : (t % 4 + 1) * P],
                ident1,
            )
        pcol = smp.tile([P, 32], F32)
        nc.vector.tensor_scalar_mul(pcol, in0=pc_ps, scalar1=r128)
        pcol_r = smp.tile([P, 32], F32R)
        nc.vector.tensor_copy(pcol_r, pcol)
        # attention weights out: attw[128 t + i] = pcol[i, t]
        nc.sync.dma_start(out=attwO[b].rearrange("(t p) -> p t", p=P), in_=pcol)

        # ---- context = sum_t p_t^T @ v_t ----
        cx_ps = ps_ms.tile([1, D], F32, tag="ms")
        for t in range(NT):
            nc.tensor.matmul(
                cx_ps,
                pcol_r[:, t : t + 1],
                nat[:, t, :],
                start=(t == 0),
                stop=(t == NT - 1),
            )
        cx_sb = smp.tile([1, D], F32)
        nc.vector.tensor_copy(cx_sb, cx_ps)
        nc.sync.dma_start(out=ctxO[b : b + 1, :], in_=cx_sb)


def _build():
    nc = bacc.Bacc(trn_type="TRN2")
    values = nc.dram_tensor("values", [BPC, S, D], F32R, kind="ExternalInput")
    query = nc.dram_tensor("query", [BPC, D], F32, kind="ExternalInput")
    W1 = nc.dram_tensor("W1", [D, U], F32, kind="ExternalInput")
    b1 = nc.dram_tensor("b1", [U], F32, kind="ExternalInput")
    W2 = nc.dram_tensor("W2", [D, U], F32, kind="ExternalInput")
    b2 = nc.dram_tensor("b2", [U], F32, kind="ExternalInput")
    V = nc.dram_tensor("V", [U, 1], F32, kind="ExternalInput")
    ctxO = nc.dram_tensor("ctx", [BPC, D], F32, kind="ExternalOutput")
    attwO = nc.dram_tensor("attw", [BPC, S], F32, kind="ExternalOutput")
    with tile.TileContext(nc) as tc, ExitStack() as ctx:
        _body(
            ctx,
            tc,
            values.ap(),
            query.ap(),
            W1.ap(),
            b1.ap(),
            W2.ap(),
            b2.ap(),
            V.ap(),
            ctxO.ap(),
            attwO.ap(),
        )
    nc.compile()
    return nc


def kernel(values, query, W1, b1, W2, b2, V, bV=None, **_ignored):
    global LAST_EXEC_NS
    values = np.ascontiguousarray(np.asarray(values, dtype=np.float32))
    query = np.ascontiguousarray(np.asarray(query, dtype=np.float32))
    W1 = np.ascontiguousarray(np.asarray(W1, dtype=np.float32))
    b1 = np.ascontiguousarray(np.asarray(b1, dtype=np.float32))
    W2 = np.ascontiguousarray(np.asarray(W2, dtype=np.float32))
    b2 = np.ascontiguousarray(np.asarray(b2, dtype=np.float32))
    V = np.ascontiguousarray(np.asarray(V, dtype=np.float32))
    # bV only shifts scores pre-softmax; softmax is shift-invariant, so it
    # does not affect either output.

    if "nc" not in _CACHED:
        _CACHED["nc"] = _build()
    nc = _CACHED["nc"]

    in_maps = []
    for i in range(NCORES):
        in_maps.append(
            {
                "values": values[BPC * i : BPC * (i + 1)],
                "query": query[BPC * i : BPC * (i + 1)],
                "W1": W1,
                "b1": b1,
                "W2": W2,
                "b2": b2,
                "V": V,
            }
        )

    kwargs = {}
    if TRACE:
        os.makedirs(TRACE_DIR, exist_ok=True)
        kwargs = dict(trace=True, tmpdir=TRACE_DIR)
    res = run_bass_kernel_spmd(nc, in_maps, core_ids=list(range(NCORES)), **kwargs)
    LAST_EXEC_NS = res.exec_time_ns

    ctx = np.concatenate([res.results[i]["ctx"] for i in range(NCORES)], axis=0)
    attw = np.concatenate([res.results[i]["attw"] for i in range(NCORES)], axis=0)
    return ctx.astype(np.float32), attw.reshape(B, S, 1).astype(np.float32)


# revision 18
# speedup vs baseline: 1.1698x; 1.1698x over previous
"""Bahdanau attention on 8 TRN2 NeuronCores (Bass/Tile).

B=16, S=4096, D=256, U=256. Data-parallel over batch: core i handles
batches [2i, 2i+2). Weights replicated.

Per-core pipeline per batch:
  1. DMA values[b] (4 MB) -> SBUF natural layout [128(s), 32(t), 256(d)].
  2. PE-transpose 128x128 blocks -> vT [128(d), s] chunks (fp32, via PSUM +
     DVE copy to SBUF).
  3. proj^T[u, s] = W1^T @ vT (fp32r matmuls, PSUM accumulate over d-halves).
  4. ACT: tanh(proj + qproj + b1 + b2) with per-partition bias (u on
     partitions), PSUM -> SBUF.
  5. score[1, s] = V^T @ tanh (fp32r), landed as PSUM rows [8, 512].
  6. Softmax without max-shift (|score| <= ||V||_1 ~ 13, exp is safe in
     fp32): ACT exp with fused row-sum, PE all-ones matmul for the
     cross-partition sum, DVE reciprocal + scale.
  7. attention weights DMA'd out; PE-transposed into per-s-tile columns
     [128, 32] for the weighted sum.
  8. context[1, 256] = sum_t p_t^T @ v_t (fp32r, PSUM accumulate), DMA out.
"""

import os
import sys
from contextlib import ExitStack

import numpy as np

if "/opt/trn_rl_repo" not in sys.path:
    sys.path.insert(0, "/opt/trn_rl_repo")

import concourse.bacc as bacc
import concourse.bass as bass
import concourse.tile as tile
from concourse import mybir
from concourse.bass_utils import run_bass_kernel_spmd
from concourse.masks import make_identity

B, S, D, U = 16, 4096, 256, 256
NCORES = 8
BPC = B // NCORES  # batches per core
P = 128
NT = S // P       # 32 s-tiles per batch
NCH = S // 512    # 8 chunks of 512 s
F32 = mybir.dt.float32
F32R = mybir.dt.float32r
AF = mybir.ActivationFunctionType

# Set by test.py to capture a profile.
TRACE = False
TRACE_DIR = "/tmp/bahdanau_trace"
LAST_EXEC_NS = None

_CACHED = {}


def _r(ap):
    return ap.bitcast(F32R)


def _body(ctx, tc, values, query, W1, b1, W2, b2, V, ctxO, attwO):
    nc = tc.nc
    consts = ctx.enter_context(tc.tile_pool(name="consts", bufs=1))
    natp = ctx.enter_context(tc.tile_pool(name="nat", bufs=1))
    vtp = ctx.enter_context(tc.tile_pool(name="vts", bufs=3))
    thp = ctx.enter_context(tc.tile_pool(name="tanh", bufs=3))
    smp = ctx.enter_context(tc.tile_pool(name="smx", bufs=2))
    ps_vt = ctx.enter_context(tc.tile_pool(name="psvt", bufs=2, space="PSUM"))
    ps_pj = ctx.enter_context(tc.tile_pool(name="pspj", bufs=2, space="PSUM"))
    ps_sc = ctx.enter_context(tc.tile_pool(name="pssc", bufs=2, space="PSUM"))
    ps_ms = ctx.enter_context(tc.tile_pool(name="psms", bufs=2, space="PSUM"))

    # ---- constants / preload ----
    # Everything the PE reads (except the big values DMA) is funneled
    # through a single DVE copy: PE matmuls carry at most ONE semaphore
    # wait (the fused-LDWEIGHTS struct only supports one), and all these
    # collapse onto the DVE clock.
    ident_st = consts.tile([P, P], F32)
    make_identity(nc, ident_st)
    w1_st = consts.tile([P, 2, U], F32)
    nc.sync.dma_start(out=w1_st, in_=W1.rearrange("(h p) u -> p h u", p=P))
    w2_st = consts.tile([P, 2, U], F32)
    nc.sync.dma_start(out=w2_st, in_=W2.rearrange("(h p) u -> p h u", p=P))
    v_st = consts.tile([P, 2], F32)
    nc.sync.dma_start(
        out=v_st, in_=V.rearrange("(h p) one -> p h one", p=P)[:, :, 0]
    )
    q_st = consts.tile([P, 2, BPC], F32)
    qr_ = query.rearrange("b (h p) -> p h b", p=P)
    for h in range(2):
        nc.sync.dma_start(out=q_st[:, h, :], in_=qr_[:, h, :])
    b1_st = consts.tile([P, 2], F32)
    nc.sync.dma_start(out=b1_st, in_=b1.rearrange("(h p) -> p h", p=P))
    b2_st = consts.tile([P, 2], F32)
    nc.sync.dma_start(out=b2_st, in_=b2.rearrange("(h p) -> p h", p=P))

    ident = consts.tile([P, P], F32R)
    nc.vector.tensor_copy(ident, ident_st)
    ident1 = consts.tile([1, 1], F32)
    nc.vector.memset(ident1, 1.0)
    ones1 = consts.tile([1, P], F32)
    nc.vector.memset(ones1, 1.0)
    w1_sb = consts.tile([P, 2, U], F32R)
    nc.vector.tensor_copy(w1_sb, w1_st)
    v_sb = consts.tile([P, 2], F32R)
    nc.vector.tensor_copy(v_sb, v_st)
    w2_sb = consts.tile([P, 2, U], F32)
    nc.vector.tensor_copy(w2_sb, w2_st)
    q_sb = consts.tile([P, 2, BPC], F32)
    for h in range(2):
        nc.vector.tensor_copy(q_sb[:, h, :], q_st[:, h, :])
    b1_sb = consts.tile([P, 2], F32)
    nc.vector.tensor_copy(b1_sb, b1_st)
    b2_sb = consts.tile([P, 2], F32)
    nc.vector.tensor_copy(b2_sb, b2_st)

    # tanh bias per (u, b): qproj[u, b] + b1[u] + b2[u]
    bias_sb = consts.tile([P, 2, BPC], F32)
    for uh in range(2):
        qp_ps = ps_ms.tile([P, BPC], F32, tag="ms")
        for dh in range(2):
            nc.tensor.matmul(
                qp_ps,
                w2_sb[:, dh, uh * P : (uh + 1) * P],
                q_sb[:, dh, :],
                start=(dh == 0),
                stop=(dh == 1),
            )
        for b in range(BPC):
            nc.vector.tensor_add(
                bias_sb[:, uh, b : b + 1], qp_ps[:, b : b + 1], b1_sb[:, uh : uh + 1]
            )
            nc.vector.tensor_add(
                bias_sb[:, uh, b : b + 1],
                bias_sb[:, uh, b : b + 1],
                b2_sb[:, uh : uh + 1],
            )

    # ---- all values DMAs up front (1 MB chunks), both batches ----
    nats = []
    for b in range(BPC):
        nat = natp.tile([P, NT, D], F32R, tag=f"nat{b}")
        vr = values[b].rearrange("(t p) d -> p t d", p=P)
        for h in range(4):
            nc.sync.dma_start(
                out=nat[:, h * 8 : (h + 1) * 8, :], in_=vr[:, h * 8 : (h + 1) * 8, :]
            )
        nats.append(nat)

    # ---- PE warmup: keep TensorE busy during the first values DMA so the
    # HAM clock gate reaches 8/8 before the real matmuls arrive ----
    warm_ps = ps_ms.tile([P, P], F32, tag="ms")
    for i in range(24):
        nc.tensor.matmul(warm_ps, ident, ident, start=True, stop=True)

    for b in range(BPC):
        nat = nats[b]
        exp_sb = smp.tile([1, NCH, 512], F32)
        zrow = smp.tile([1, NCH], F32)
        pcol_raw = smp.tile([P, NT], F32)
        for c in range(NCH):
            # ---- transpose 512 s-values x 256 d into vT chunk ----
            vt_sb = vtp.tile([P, 2, 512], F32R)
            for q in range(2):
                vt_ps = ps_vt.tile([P, 2, 256], F32R)
                for dh in range(2):
                    for j in range(2):
                        t = 4 * c + 2 * q + j
                        nc.tensor.transpose(
                            vt_ps[:, dh, j * P : (j + 1) * P],
                            nat[:, t, dh * P : (dh + 1) * P],
                            ident,
                        )
                nc.vector.tensor_copy(vt_sb[:, :, q * 256 : (q + 1) * 256], vt_ps)

            # ---- proj + tanh + score for this chunk ----
            score_ps = ps_sc.tile([1, 512], F32)
            for uh in range(2):
                pj = ps_pj.tile([P, 512], F32)
                for dh in range(2):
                    nc.tensor.matmul(
                        pj,
                        w1_sb[:, dh, uh * P : (uh + 1) * P],
                        vt_sb[:, dh, :],
                        start=(dh == 0),
                        stop=(dh == 1),
                    )
                th = thp.tile([P, 512], F32R)
                nc.scalar.activation(th, pj, AF.Tanh, bias=bias_sb[:, uh, b : b + 1])
                nc.tensor.matmul(
                    score_ps,
                    v_sb[:, uh : uh + 1],
                    _r(th),
                    start=(uh == 0),
                    stop=(uh == 1),
                )
            # exp with fused row-sum; no max-shift (scores bounded by ||V||_1)
            nc.scalar.activation(
                exp_sb[:, c, :],
                score_ps,
                AF.Exp,
                accum_out=zrow[:, c : c + 1],
            )
            # unnormalized weights -> per-s-tile columns, while later chunks
            # are still computing
            pcq_ps = ps_ms.tile([P, 4], F32, tag="ms")
            for k in range(4):
                nc.tensor.transpose(
                    pcq_ps[:, k : k + 1],
                    exp_sb[:, c, k * P : (k + 1) * P],
                    ident1,
                )
            nc.vector.tensor_copy(pcol_raw[:, 4 * c : 4 * (c + 1)], pcq_ps)

        # Z = sum of row sums; 1/Z broadcast to all 128 partitions via k=1
        # all-ones matmul.
        z_sb = smp.tile([1, 1], F32)
        nc.vector.reduce_sum(z_sb, zrow, axis=mybir.AxisListType.X)
        zb_ps = ps_ms.tile([P, 1], F32, tag="ms")
        nc.tensor.matmul(zb_ps, ones1[:1, :], z_sb, start=True, stop=True)
        r128 = smp.tile([P, 1], F32)
        nc.vector.reciprocal(r128, zb_ps)

        # normalized attention weights, contiguous in s on one partition ->
        # cheap contiguous DMA out
        wn = smp.tile([1, NCH * 512], F32)
        nc.scalar.activation(
            wn,
            exp_sb.rearrange("one c s -> one (c s)"),
            AF.Copy,
            scale=r128[0:1, :],
        )
        nc.sync.dma_start(out=attwO[b : b + 1, :], in_=wn)

        pcol_r = smp.tile([P, NT], F32R)
        nc.vector.tensor_scalar_mul(pcol_r, in0=pcol_raw, scalar1=r128)

        # ---- context = sum_t p_t^T @ v_t ----
        cx_ps = ps_ms.tile([1, D], F32, tag="ms")
        for t in range(NT):
            nc.tensor.matmul(
                cx_ps,
                pcol_r[:, t : t + 1],
                nat[:, t, :],
                start=(t == 0),
                stop=(t == NT - 1),
            )
        cx_sb = smp.tile([1, D], F32)
        nc.vector.tensor_copy(cx_sb, cx_ps)
        nc.sync.dma_start(out=ctxO[b : b + 1, :], in_=cx_sb)


def _build():
    nc = bacc.Bacc(trn_type="TRN2")
    values = nc.dram_tensor("values", [BPC, S, D], F32R, kind="ExternalInput")
    query = nc.dram_tensor("query", [BPC, D], F32, kind="ExternalInput")
    W1 = nc.dram_tensor("W1", [D, U], F32, kind="ExternalInput")
    b1 = nc.dram_tensor("b1", [U], F32, kind="ExternalInput")
    W2 = nc.dram_tensor("W2", [D, U], F32, kind="ExternalInput")
    b2 = nc.dram_tensor("b2", [U], F32, kind="ExternalInput")
    V = nc.dram_tensor("V", [U, 1], F32, kind="ExternalInput")
    ctxO = nc.dram_tensor("ctx", [BPC, D], F32, kind="ExternalOutput")
    attwO = nc.dram_tensor("attw", [BPC, S], F32, kind="ExternalOutput")
    with tile.TileContext(nc) as tc, ExitStack() as ctx:
        _body(
            ctx,
            tc,
            values.ap(),
            query.ap(),
            W1.ap(),
            b1.ap(),
            W2.ap(),
            b2.ap(),
            V.ap(),
            ctxO.ap(),
            attwO.ap(),
        )
    nc.compile()
    return nc


def kernel(values, query, W1, b1, W2, b2, V, bV=None, **_ignored):
    global LAST_EXEC_NS
    values = np.ascontiguousarray(np.asarray(values, dtype=np.float32))
    query = np.ascontiguousarray(np.asarray(query, dtype=np.float32))
    W1 = np.ascontiguousarray(np.asarray(W1, dtype=np.float32))
    b1 = np.ascontiguousarray(np.asarray(b1, dtype=np.float32))
    W2 = np.ascontiguousarray(np.asarray(W2, dtype=np.float32))
    b2 = np.ascontiguousarray(np.asarray(b2, dtype=np.float32))
    V = np.ascontiguousarray(np.asarray(V, dtype=np.float32))
    # bV only shifts scores pre-softmax; softmax is shift-invariant, so it
    # does not affect either output.

    if "nc" not in _CACHED:
        _CACHED["nc"] = _build()
    nc = _CACHED["nc"]

    in_maps = []
    for i in range(NCORES):
        in_maps.append(
            {
                "values": values[BPC * i : BPC * (i + 1)],
                "query": query[BPC * i : BPC * (i + 1)],
                "W1": W1,
                "b1": b1,
                "W2": W2,
                "b2": b2,
                "V": V,
            }
        )

    kwargs = {}
    if TRACE:
        import shutil

        shutil.rmtree(TRACE_DIR, ignore_errors=True)
        os.makedirs(TRACE_DIR, exist_ok=True)
        kwargs = dict(trace=True, tmpdir=TRACE_DIR)
    res = run_bass_kernel_spmd(nc, in_maps, core_ids=list(range(NCORES)), **kwargs)
    LAST_EXEC_NS = res.exec_time_ns

    ctx = np.concatenate([res.results[i]["ctx"] for i in range(NCORES)], axis=0)
    attw = np.concatenate([res.results[i]["attw"] for i in range(NCORES)], axis=0)
    return ctx.astype(np.float32), attw.reshape(B, S, 1).astype(np.float32)
